# revision 1
# baseline (speedup 1.0000x reference)
"""ConcatAttention Trainium2 kernel (8-core data-parallel over batch).

Computes, per batch row b:
    scores[b, l] = sum_h v[h] * tanh(q_proj[b, h] + (key_val[l, b] @ Wk)[h])
    out[b, 0, :] = softmax(scores[b, :])

Device-side per core (B_shard = 4 batch rows), partitions = h orientation:
  - main matmul  kpT[h, l] = Wk^T @ keyT   (f32r, K=512 via 4 PSUM-accum chunks)
  - ACT fuses    en_ch = tanh(kpT + q_projT[h])  (per-partition bias, exact)
  - DVE combine  vac[p, l] = sum_ch v[ch,p] * en_ch[p, l]
                 (4 tensor_scalar muls @2x_2p + 3 tensor_tensor adds --
                  replaces 3/4 of the old PE v-dot matmul rows)
  - PE ones-matmul [1, L-tile] reduces vac over partitions -> raw scores
  - raw scores DMA straight from PSUM to DRAM; softmax done on host
    (host already computes q_proj; normalizing [B, L] is trivial numpy).
"""

import os
import sys

for _p in ("/opt/trn_rl_repo", os.path.expanduser("~/trn_rl_repo")):
    if os.path.isdir(_p) and _p not in sys.path:
        sys.path.insert(0, _p)

import numpy as np

L, B, H = 4096, 32, 512
NCORES = 8
BS = B // NCORES          # batch rows per core
P = 128
CI = H // P               # input-feature chunks (contraction)
CH = H // P               # output-feature chunks
LC = 512                  # l-tile (matmul moving free dim)
NLC = L // LC
QRT = L // 4              # key DMA granularity: [128, QRT] = 512 KiB
WARMUP_MM = 4             # dummy matmul groups to heat the PE HAM clock gate

_CACHE = {}


def _build_nc():
    import concourse.bacc as bacc
    import concourse.mybir as mybir
    import concourse.tile as tile

    f32 = mybir.dt.float32
    f32r = mybir.dt.float32r
    f16 = mybir.dt.float16
    Act = mybir.ActivationFunctionType
    Alu = mybir.AluOpType

    nc = bacc.Bacc("TRN2", target_bir_lowering=False)

    keyT = nc.dram_tensor("keyT", [BS, CI, P, L], f32r, kind="ExternalInput")
    wk = nc.dram_tensor("wk", [P, CI, H], f32r, kind="ExternalInput")
    qpT = nc.dram_tensor("qpT", [P, CH, BS], f32, kind="ExternalInput")
    vT = nc.dram_tensor("vT", [P, CH], f32, kind="ExternalInput")
    onesd = nc.dram_tensor("onesd", [P, 1], f16, kind="ExternalInput")
    out = nc.dram_tensor("out", [BS, L], f32, kind="ExternalOutput")

    with tile.TileContext(nc) as tc:
        with tc.tile_pool(name="singles", bufs=1) as singles, \
             tc.tile_pool(name="ktp", bufs=8) as ktp, \
             tc.tile_pool(name="enp", bufs=10) as enp, \
             tc.tile_pool(name="vacp", bufs=14) as vacp, \
             tc.tile_pool(name="kpp", bufs=6, space="PSUM") as kpp, \
             tc.tile_pool(name="scp", bufs=2, space="PSUM") as scp:

            def load_kt(b, plan, tiles=None, pos=0, queues=None):
                """plan: list of l-slice widths; each slice is one joint DMA
                carrying all CI feature chunks. queues: optional list of
                engines to spread the DMAs across (round-robin)."""
                if tiles is None:
                    tiles = []
                for gi, w in enumerate(plan):
                    t = ktp.tile([P, CI, QRT], f32r, tag="kt")
                    eng = nc.sync if queues is None else queues[gi % len(queues)]
                    eng.dma_start(
                        t[:, :, :w],
                        keyT[b, :, :, pos:pos + w].rearrange("c p l -> p c l"))
                    tiles.append((pos, w, t))
                    pos += w
                return tiles

            def kt_slice(tiles, ci, l0):
                for pos, w, t in tiles:
                    if pos <= l0 and l0 + LC <= pos + w:
                        return t[:, ci, l0 - pos:l0 - pos + LC]
                raise AssertionError("no tile covers slice")

            # ---- constants on the gpsimd queue (per-ci so the first matmul
            # group can start as soon as its chunk lands) ----
            wk_sb = singles.tile([P, CI, H], f32r, tag="wk")
            for ci in range(CI):
                nc.gpsimd.dma_start(wk_sb[:, ci, :], wk[:, ci, :])
            qpT_sb = singles.tile([P, CH, BS], f32, tag="qpT")
            nc.gpsimd.dma_start(qpT_sb, qpT[:, :, :])
            vT_sb = singles.tile([P, CH], f32, tag="vT")
            nc.gpsimd.dma_start(vT_sb, vT[:, :])
            ones = singles.tile([P, 1], f16, tag="ones")
            nc.gpsimd.dma_start(ones, onesd[:, :])
            ones_r = ones[:, :]
            kts = load_kt(0, [LC, LC, LC, LC, QRT, QRT],
                          queues=[nc.sync, nc.scalar])

            # ---- PE warmup: cheap dummy matmuls on zeros while the first
            # key tiles stream in, so the HAM clock gate reaches 2.4 GHz
            # before real work starts ----
            wu = singles.tile([P, LC], f32, tag="warmup")
            nc.vector.memset(wu, 0.0)
            wur = wu[:, :].bitcast(f32r)
            trash = singles.tile([1, 1], f32, tag="trash")
            for g in range(WARMUP_MM):
                wps = kpp.tile([P, LC], f32, tag="kp")
                for i in range(4):
                    nc.tensor.matmul(wps[:, 0:P], wur[:, 0:P], wur[:, 0:P],
                                     start=(i == 0), stop=(i == 3))
                nc.vector.tensor_copy(trash, wps[0:1, 0:1])

            def finish(vacr, b, lc):
                # partition-reduce on PE: scores[1, LC] = ones^T @ vacr,
                # PSUM -> SBUF on ACT, then to DRAM (softmax on host)
                sc = scp.tile([1, LC], f32, tag="sc")
                nc.tensor.matmul(sc, ones_r, vacr[:, :],
                                 start=True, stop=True)
                scs = vacp.tile([1, LC], f32, tag="scs")
                nc.scalar.copy(scs, sc)
                nc.sync.dma_start(out[b:b + 1, lc * LC:(lc + 1) * LC], scs)

            pending = None  # (vacr, b, lc) awaiting the PE ones-matmul
            for b in range(BS):
                for lc in range(NLC):
                    ens = []
                    for ch in range(CH):
                        ps = kpp.tile([P, LC], f32, tag="kp")
                        for ci in range(CI):
                            nc.tensor.matmul(
                                ps,
                                wk_sb[:, ci, ch * P:(ch + 1) * P],
                                kt_slice(kts, ci, lc * LC),
                                start=(ci == 0), stop=(ci == CI - 1))
                        en = enp.tile([P, LC], f16, tag="en")
                        nc.scalar.activation(en, ps, Act.Tanh,
                                             bias=qpT_sb[:, ch, b:b + 1])
                        ens.append(en)
                    # software-pipeline: previous chunk's ones-matmul lands
                    # after this chunk's main matmuls so PE never waits on
                    # the DVE combine latency.
                    if pending is not None:
                        finish(*pending)
                    # DVE combine in fp16: vac = sum_ch v[ch] * en_ch
                    # (TS-mul hits 4x_2p, TT-add 2x_1p; tree (m0+m1)+(m2+m3))
                    vac = vacp.tile([P, LC], f16, tag="vac")
                    tmp = vacp.tile([P, LC], f16, tag="vac")
                    nc.vector.tensor_scalar_mul(vac, in0=ens[0],
                                                scalar1=vT_sb[:, 0:1])
                    nc.vector.tensor_scalar_mul(tmp, in0=ens[1],
                                                scalar1=vT_sb[:, 1:2])
                    nc.vector.tensor_tensor(out=vac, in0=vac, in1=tmp,
                                            op=Alu.add)
                    tmp2 = vacp.tile([P, LC], f16, tag="vac")
                    tmp3 = vacp.tile([P, LC], f16, tag="vac")
                    nc.vector.tensor_scalar_mul(tmp2, in0=ens[2],
                                                scalar1=vT_sb[:, 2:3])
                    nc.vector.tensor_scalar_mul(tmp3, in0=ens[3],
                                                scalar1=vT_sb[:, 3:4])
                    nc.vector.tensor_tensor(out=tmp2, in0=tmp2, in1=tmp3,
                                            op=Alu.add)
                    vacr = vacp.tile([P, LC], f16, tag="vac")
                    nc.vector.tensor_tensor(out=vacr, in0=vac, in1=tmp2,
                                            op=Alu.add)
                    pending = (vacr, b, lc)
                    if lc == 2 and b + 1 < BS:
                        next_kts = load_kt(b + 1, [QRT] * 4)
                if b + 1 < BS:
                    kts = next_kts
            finish(*pending)

    nc.compile()
    return nc


def _get_nc():
    if "nc" not in _CACHE:
        _CACHE["nc"] = _build_nc()
    return _CACHE["nc"]


def _prep_inputs(query, key_val, W, v):
    """Host-side shard prep: returns list of 8 per-core input dicts."""
    query = np.asarray(query, dtype=np.float32)
    key_val = np.asarray(key_val, dtype=np.float32)
    W = np.asarray(W, dtype=np.float32)
    v = np.asarray(v, dtype=np.float32)

    q_proj = (query.astype(np.float64)
              @ W[:H].astype(np.float64)).astype(np.float32)
    wk_tiled = np.ascontiguousarray(
        W[H:].reshape(CI, P, H).transpose(1, 0, 2))          # [P, CI, H]
    vT_tiled = np.ascontiguousarray(v.reshape(CH, P).T)      # [P, CH]

    in_maps = []
    for c in range(NCORES):
        b0 = c * BS
        # key_val[l, b, i] -> [b, ci, p(i), l]
        kt = np.ascontiguousarray(
            key_val[:, b0:b0 + BS, :].transpose(1, 2, 0)
            .reshape(BS, CI, P, L))
        qpT_tiled = np.ascontiguousarray(
            q_proj[b0:b0 + BS].T.reshape(CH, P, BS).transpose(1, 0, 2))
        in_maps.append({
            "keyT": kt,
            "wk": wk_tiled,
            "qpT": qpT_tiled,
            "vT": vT_tiled,
            "onesd": np.ones((P, 1), np.float16),
        })
    return in_maps


def _run(inputs, trace=False, **trace_kwargs):
    from concourse.bass_utils import run_bass_kernel_spmd

    nc = _get_nc()
    in_maps = _prep_inputs(**inputs)
    res = run_bass_kernel_spmd(
        nc, in_maps, core_ids=list(range(NCORES)), trace=trace, **trace_kwargs)
    scores = np.concatenate(
        [np.asarray(r["out"], dtype=np.float32) for r in res.results],
        axis=0)                                              # (B, L)
    # softmax on host (float64)
    s = scores.astype(np.float64)
    s -= s.max(axis=1, keepdims=True)
    e = np.exp(s)
    p = e / e.sum(axis=1, keepdims=True)
    return p.astype(np.float32).reshape(B, 1, L), res


def kernel(**inputs):
    out, _ = _run(inputs, trace=False)
    return out



# revision 3
# speedup vs baseline: 1.3609x; 1.3609x over previous
"""ConcatAttention Trainium2 kernel (8-core data-parallel over batch).

Computes, per batch row b:
    scores[b, l] = sum_h v[h] * tanh(q_proj[b, h] + (key_val[l, b] @ Wk)[h])
    out[b, 0, :] = softmax(scores[b, :])

v2: fp16 key/weight matmuls (f32r HIGH mode measured ~281ns per 512-row
matmul; fp16 should stream 1 row/cycle and halves the key DMA bytes),
LC=1024 ACT/DVE tiles to amortize the ~400ns per-instruction overhead
on the activation engine (tanh bias is per-partition, constant along l,
so wider l-tiles are free), score copies kept on ACT (it has slack now).

Device-side per core (B_shard = 4 batch rows), partitions = h:
  - main matmul  kpT[h, l] = Wk^T @ keyT  (f16, K=512 via 4 PSUM-accum
    chunks, N=512 per PSUM bank, 2 banks per LC=1024 tile)
  - ACT fuses    en_ch = tanh(kpT + q_projT[h])  (per-partition bias)
  - DVE combine  vac[p, l] = sum_ch v[ch,p] * en_ch[p, l]
  - PE ones-matmul [1, 512] reduces vac over partitions -> raw scores
  - raw scores PSUM -> SBUF on ACT, DMA to DRAM; softmax on host.
"""

import os
import sys

for _p in ("/opt/trn_rl_repo", os.path.expanduser("~/trn_rl_repo")):
    if os.path.isdir(_p) and _p not in sys.path:
        sys.path.insert(0, _p)

import numpy as np

L, B, H = 4096, 32, 512
NCORES = 8
BS = B // NCORES          # batch rows per core
P = 128
CI = H // P               # input-feature chunks (contraction)
CH = H // P               # output-feature chunks
LC = 1024                 # l-tile for ACT/DVE
NB = 512                  # matmul moving window (one PSUM bank of f32)
NLC = L // LC
QRT = 1024                # key DMA granularity: [128, CI, QRT] f16 = 1 MiB
WARMUP_MM = 4             # dummy matmul groups to heat the PE clock gate

_CACHE = {}


def _build_nc():
    import concourse.bacc as bacc
    import concourse.mybir as mybir
    import concourse.tile as tile

    f32 = mybir.dt.float32
    f16 = mybir.dt.float16
    Act = mybir.ActivationFunctionType
    Alu = mybir.AluOpType

    nc = bacc.Bacc("TRN2", target_bir_lowering=False)

    keyT = nc.dram_tensor("keyT", [BS, CI, P, L], f16, kind="ExternalInput")
    wk = nc.dram_tensor("wk", [P, CI, H], f16, kind="ExternalInput")
    qpT = nc.dram_tensor("qpT", [P, CH, BS], f32, kind="ExternalInput")
    vT = nc.dram_tensor("vT", [P, CH], f32, kind="ExternalInput")
    onesd = nc.dram_tensor("onesd", [P, 1], f16, kind="ExternalInput")
    out = nc.dram_tensor("out", [BS, L], f32, kind="ExternalOutput")

    with tile.TileContext(nc) as tc:
        with tc.tile_pool(name="singles", bufs=1) as singles, \
             tc.tile_pool(name="ktp", bufs=8) as ktp, \
             tc.tile_pool(name="enp", bufs=6) as enp, \
             tc.tile_pool(name="vacp", bufs=10) as vacp, \
             tc.tile_pool(name="scsp", bufs=4) as scsp, \
             tc.tile_pool(name="kpp", bufs=3, space="PSUM") as kpp, \
             tc.tile_pool(name="scp", bufs=2, space="PSUM") as scp:

            def load_kt(b, tiles=None, queues=(nc.sync,), split_ci=False):
                """Load all of keyT[b] as NLC window tiles of [P, CI, QRT].
                split_ci: issue one DMA per ci chunk (faster first-use)."""
                if tiles is None:
                    tiles = []
                for gi in range(L // QRT):
                    pos = gi * QRT
                    t = ktp.tile([P, CI, QRT], f16, tag="kt")
                    eng = queues[gi % len(queues)]
                    if split_ci and gi == 0:
                        for ci in range(CI):
                            eng.dma_start(
                                t[:, ci, :],
                                keyT[b, ci, :, pos:pos + QRT])
                    else:
                        eng.dma_start(
                            t[:, :, :],
                            keyT[b, :, :, pos:pos + QRT]
                            .rearrange("c p l -> p c l"))
                    tiles.append((pos, QRT, t))
                return tiles

            def kt_slice(tiles, ci, l0, w):
                for pos, tw, t in tiles:
                    if pos <= l0 and l0 + w <= pos + tw:
                        return t[:, ci, l0 - pos:l0 - pos + w]
                raise AssertionError("no tile covers slice")

            # ---- constants on the gpsimd queue (per-ci so the first matmul
            # group can start as soon as its chunk lands) ----
            wk_sb = singles.tile([P, CI, H], f16, tag="wk")
            for ci in range(CI):
                nc.gpsimd.dma_start(wk_sb[:, ci, :], wk[:, ci, :])
            qpT_sb = singles.tile([P, CH, BS], f32, tag="qpT")
            nc.gpsimd.dma_start(qpT_sb, qpT[:, :, :])
            vT_sb = singles.tile([P, CH], f32, tag="vT")
            nc.gpsimd.dma_start(vT_sb, vT[:, :])
            ones = singles.tile([P, 1], f16, tag="ones")
            nc.gpsimd.dma_start(ones, onesd[:, :])
            ones_r = ones[:, :]
            kts = load_kt(0, queues=[nc.sync, nc.gpsimd], split_ci=True)

            # ---- PE warmup: cheap dummy matmuls on zeros while the first
            # key tiles stream in, so the clock gate ramps before real
            # work starts ----
            wu = singles.tile([P, NB], f16, tag="warmup")
            nc.vector.memset(wu, 0.0)
            trash = singles.tile([1, 1], f32, tag="trash")
            for g in range(WARMUP_MM):
                wps = scp.tile([1, NB], f32, tag="sc")
                for i in range(4):
                    nc.tensor.matmul(wps, wu[:, 0:1], wu[:, :],
                                     start=(i == 0), stop=(i == 3))
                nc.vector.tensor_copy(trash, wps[0:1, 0:1])

            def finish(vacr, b, lc):
                # partition-reduce on PE: scores[1, NB] = ones^T @ vacr half,
                # PSUM -> SBUF on ACT, then to DRAM (softmax on host)
                for h in range(LC // NB):
                    sc = scp.tile([1, NB], f32, tag="sc")
                    nc.tensor.matmul(sc, ones_r,
                                     vacr[:, h * NB:(h + 1) * NB],
                                     start=True, stop=True)
                    scs = scsp.tile([1, NB], f32, tag="scs")
                    nc.scalar.copy(scs, sc)
                    l0 = lc * LC + h * NB
                    nc.sync.dma_start(out[b:b + 1, l0:l0 + NB], scs)

            pending = None  # (vacr, b, lc) awaiting the PE ones-matmul
            for b in range(BS):
                for lc in range(NLC):
                    ens = []
                    for ch in range(CH):
                        ps = kpp.tile([P, LC], f32, tag="kp")
                        for ci in range(CI):
                            for j in range(2):
                                nc.tensor.matmul(
                                    ps[:, j * NB:(j + 1) * NB],
                                    wk_sb[:, ci, ch * P:(ch + 1) * P],
                                    kt_slice(kts, ci, lc * LC + j * NB, NB),
                                    start=(ci == 0), stop=(ci == CI - 1))
                        en = enp.tile([P, LC], f16, tag="en")
                        nc.scalar.activation(en, ps[:, :], Act.Tanh,
                                             bias=qpT_sb[:, ch, b:b + 1])
                        ens.append(en)
                    # software-pipeline: previous tile's ones-matmuls land
                    # after this tile's main matmuls so PE never waits on
                    # the DVE combine latency.
                    if pending is not None:
                        finish(*pending)
                    # DVE combine in fp16: vac = sum_ch v[ch] * en_ch
                    # (TS-mul hits 4x_2p, TT-add 2x_1p; tree (m0+m1)+(m2+m3))
                    vac = vacp.tile([P, LC], f16, tag="vac")
                    tmp = vacp.tile([P, LC], f16, tag="vac")
                    nc.vector.tensor_scalar_mul(vac, in0=ens[0],
                                                scalar1=vT_sb[:, 0:1])
                    nc.vector.tensor_scalar_mul(tmp, in0=ens[1],
                                                scalar1=vT_sb[:, 1:2])
                    nc.vector.tensor_tensor(out=vac, in0=vac, in1=tmp,
                                            op=Alu.add)
                    tmp2 = vacp.tile([P, LC], f16, tag="vac")
                    tmp3 = vacp.tile([P, LC], f16, tag="vac")
                    nc.vector.tensor_scalar_mul(tmp2, in0=ens[2],
                                                scalar1=vT_sb[:, 2:3])
                    nc.vector.tensor_scalar_mul(tmp3, in0=ens[3],
                                                scalar1=vT_sb[:, 3:4])
                    nc.vector.tensor_tensor(out=tmp2, in0=tmp2, in1=tmp3,
                                            op=Alu.add)
                    vacr = vacp.tile([P, LC], f16, tag="vac")
                    nc.vector.tensor_tensor(out=vacr, in0=vac, in1=tmp2,
                                            op=Alu.add)
                    pending = (vacr, b, lc)
                    if lc == 1 and b + 1 < BS:
                        next_kts = load_kt(b + 1, queues=[nc.sync, nc.gpsimd])
                if b + 1 < BS:
                    kts = next_kts
            finish(*pending)

    nc.compile()
    return nc


def _get_nc():
    if "nc" not in _CACHE:
        _CACHE["nc"] = _build_nc()
    return _CACHE["nc"]


def _prep_inputs(query, key_val, W, v):
    """Host-side shard prep: returns list of 8 per-core input dicts."""
    query = np.asarray(query, dtype=np.float32)
    key_val = np.asarray(key_val, dtype=np.float32)
    W = np.asarray(W, dtype=np.float32)
    v = np.asarray(v, dtype=np.float32)

    q_proj = (query.astype(np.float64)
              @ W[:H].astype(np.float64)).astype(np.float32)
    wk_tiled = np.ascontiguousarray(
        W[H:].reshape(CI, P, H).transpose(1, 0, 2)).astype(np.float16)
    vT_tiled = np.ascontiguousarray(v.reshape(CH, P).T)      # [P, CH]

    in_maps = []
    for c in range(NCORES):
        b0 = c * BS
        # key_val[l, b, i] -> [b, ci, p(i), l]
        kt = np.ascontiguousarray(
            key_val[:, b0:b0 + BS, :].transpose(1, 2, 0)
            .reshape(BS, CI, P, L)).astype(np.float16)
        qpT_tiled = np.ascontiguousarray(
            q_proj[b0:b0 + BS].T.reshape(CH, P, BS).transpose(1, 0, 2))
        in_maps.append({
            "keyT": kt,
            "wk": wk_tiled,
            "qpT": qpT_tiled,
            "vT": vT_tiled,
            "onesd": np.ones((P, 1), np.float16),
        })
    return in_maps


def _run(inputs, trace=False, **trace_kwargs):
    from concourse.bass_utils import run_bass_kernel_spmd

    nc = _get_nc()
    in_maps = _prep_inputs(**inputs)
    res = run_bass_kernel_spmd(
        nc, in_maps, core_ids=list(range(NCORES)), trace=trace, **trace_kwargs)
    scores = np.concatenate(
        [np.asarray(r["out"], dtype=np.float32) for r in res.results],
        axis=0)                                              # (B, L)
    # softmax on host (float64)
    s = scores.astype(np.float64)
    s -= s.max(axis=1, keepdims=True)
    e = np.exp(s)
    p = e / e.sum(axis=1, keepdims=True)
    return p.astype(np.float32).reshape(B, 1, L), res


def kernel(**inputs):
    out, _ = _run(inputs, trace=False)
    return out
